# revision 2
# baseline (speedup 1.0000x reference)
"""Trainium2 Bass kernel for nn_DLGN_VT (deep linearly-gated network w/ value tensor).

Math (per batch row b):
    g_i = sigmoid(30 * x @ W_i.T)            i = 1,2,3    [B, 32] each
    out[b] = sum_{ijk} g1[b,i] g2[b,j] g3[b,k] V[i,j,k]

Distribution: pure data-parallel over the batch axis, 8 NeuronCores,
512 rows per core. W_i and V are tiny and replicated.

Per-core v2 schedule (v1 was 26.0us; profile-driven rework):
  - Inputs split across 3 DMA queues (SP + ACT HW-DGE, Pool SW-DGE) so the
    x shard lands ~2us earlier than v1's single-queue load:
      SP:  xa0 [64, 704]  = rows 0:64  of xh|wh|wl   then xb0 [64,512] (xl)
      ACT: xa1 [64, 704]  = rows 64:128               then xb1
      PL:  cb  [128, 384] = V^T chunks | S3           then s2 [32, 1025]
  - ~18 short (N=128) warmup matmuls keep the PE HAM activity window busy
    during the DMA wait with <=130ns overshoot granularity.
  - Error-compensated bf16 gating (Wh.xh + Wl.xh + Wh.xl ordered so the two
    xa-only passes run before xb arrives): Gps[96, 512] fp32-grade.
  - sigmoid -> g2t/g3t bf16; E3 = S3.T@g3t -> e3s bf16 (copy split DVE/ACT,
    emitted BEFORE the g1 sigmoid so TT0 isn't delayed behind it).
  - A^T pair-blocks: E2 selection matmuls -> PSUM; pairs 0,1: DVE
    tensor_tensor straight from PSUM (1x mode); pairs 2,3: ACT copies the
    PSUM pair to SBUF bf16 first so the DVE TT runs in 2x mode (~690 vs
    1224ns) while ACT is otherwise idle.
  - C^T accumulates over 8 bf16 matmuls (V^T chunks stationary).
  - out = ones.T @ (g1t .* C^T); final copy split ACT/DVE; single-packet
    output DMA.
"""

import numpy as np
import ml_dtypes

import concourse.bass as bass
import concourse.bacc as bacc
import concourse.mybir as mybir
import concourse.tile as tile
from concourse.alu_op_type import AluOpType
from concourse.bass_utils import run_bass_kernel_spmd

BF16 = ml_dtypes.bfloat16
NCORES = 8
B, D, N = 4096, 128, 32
BL = B // NCORES  # 512 batch rows per core
BETA = 30.0
NQ = 8   # 128-row blocks of the jk=1024 plane
NP = 4   # pairs of blocks

F32 = mybir.dt.float32
DBF = mybir.dt.bfloat16

# xa: packed bf16 [128, 704]: xh | wh | wl   (bf16 hi of xT shard; hi/lo of
# Wall^T with order W2;W3;W1).  Loaded as two 64-partition halves on two
# queues.  xb: [128, 512] = xl (bf16 lo of xT shard).
XH0, XH1 = 0, BL
WH0, WH1 = XH1, XH1 + 96
WL0, WL1 = WH1, WH1 + 96
XA1 = WL1  # 704
# cb: bf16 [128, 384]: V^T chunks [128, 256] | S3 at rows 32:64, cols 256:384
VT0, VT1 = 0, 256
S30, S31 = 256, 384
# s2: bf16 [32, 1025]: S2 selections [32, 1024] | ones [32, 1]
S20, S21 = 0, 1024
ON0 = 1024

N_WARMUP = 18   # short N=128 matmuls; ~130ns granularity during DMA wait
NW = 128        # warmup matmul free dim
N_SBUF_PAIRS = 2  # pairs whose TT runs 2x from ACT-copied SBUF bf16


def build_nc():
    # Bacc (not raw Bass): its compile passes split multi-wait sync infos
    # (TRN2 allows at most one sync wait per compute instruction).
    nc = bacc.Bacc(None)
    xa0_d = nc.declare_dram_parameter("xa0", [64, XA1], DBF, isOutput=False)
    xa1_d = nc.declare_dram_parameter("xa1", [64, XA1], DBF, isOutput=False)
    xb0_d = nc.declare_dram_parameter("xb0", [64, BL], DBF, isOutput=False)
    xb1_d = nc.declare_dram_parameter("xb1", [64, BL], DBF, isOutput=False)
    cb_d = nc.declare_dram_parameter("cb", [128, 384], DBF, isOutput=False)
    s2_d = nc.declare_dram_parameter("s2", [32, 1025], DBF, isOutput=False)
    out_d = nc.declare_dram_parameter("out", [1, BL], F32, isOutput=True)

    sig = mybir.ActivationFunctionType.Sigmoid

    with tile.TileContext(nc) as tc:
        with (
            tc.tile_pool(name="const", bufs=1) as cpool,
            tc.tile_pool(name="work", bufs=1) as wpool,
            tc.tile_pool(name="atp", bufs=1) as apool,
            tc.tile_pool(name="psA", bufs=2, space="PSUM") as psA,
            tc.tile_pool(name="psB", bufs=3, space="PSUM") as psB,
        ):
            xa = cpool.tile([128, XA1], DBF)
            xb = cpool.tile([128, BL], DBF)
            cb = cpool.tile([128, 384], DBF)
            s2t = cpool.tile([32, 1025], DBF)

            # ---- input DMAs, three parallel queues; xa (gating passes 1+2)
            # first on the two HW-DGE queues ----
            nc.sync.dma_start(xa[0:64, :], xa0_d[:])
            nc.scalar.dma_start(xa[64:128, :], xa1_d[:])
            nc.sync.dma_start(xb[0:64, :], xb0_d[:])
            nc.scalar.dma_start(xb[64:128, :], xb1_d[:])
            nc.gpsimd.dma_start(cb[:], cb_d[:])
            nc.gpsimd.dma_start(s2t[:], s2_d[:])

            xh = xa[:, XH0:XH1]
            wh = xa[:, WH0:WH1]
            wl = xa[:, WL0:WL1]
            xl = xb[:]
            vts = cb[:, VT0:VT1]             # [128, 8*32] V^T chunks (C lhsT)
            s3 = cb[32:64, S30:S31]          # [32, 128] E3 selection (base 32)
            s2 = s2t[:, S20:S21]             # [32, 8*128] E2 selection blocks
            ones = s2t[:, ON0 : ON0 + 1]     # [32, 1]

            # ---- PE warmup in the gating PSUM bank (overwritten later).
            # Short matmuls: fine-grained HAM busy-keeping, small overshoot
            # when the xa DMA lands. memset on DVE (idle until e3 copy). ----
            gps = psA.tile([96, BL], F32, tag="ps")
            wz = wpool.tile([128, NW], DBF)
            nc.vector.memset(wz[:], 0.0)
            for _ in range(N_WARMUP):
                nc.tensor.matmul(gps[:, 0:NW], wz[:, 0:96], wz[:],
                                 start=True, stop=True)

            # ---- gating: error-compensated bf16 matmul; the two passes
            # needing only xa run first so only pass 3 waits for xl ----
            nc.tensor.matmul(gps[:], wh, xh, start=True, stop=False)
            nc.tensor.matmul(gps[:], wl, xh, start=False, stop=False)
            nc.tensor.matmul(gps[:], wh, xl, start=False, stop=True)

            g23 = wpool.tile([2 * N, BL], DBF)
            g1t = wpool.tile([N, BL], F32)
            nc.scalar.activation(g23[:], gps[0:64, :], sig, scale=BETA)
            g2t = g23[0:32, :]
            g3t = g23[32:64, :]  # base partition 32, matching s3

            # ---- E3 = S3.T @ g3t -> SBUF bf16. The PSUM->SBUF copy gates
            # TT0, so split across DVE and ACT; g1's sigmoid is emitted
            # AFTER so the scheduler can't hoist it in front. ----
            e3ps = psA.tile([128, BL], F32, tag="ps")
            nc.tensor.matmul(e3ps[:], s3, g3t, start=True, stop=True)
            e3s = wpool.tile([128, BL], DBF)
            HB = BL // 2
            nc.vector.tensor_copy(e3s[:, 0:HB], e3ps[:, 0:HB])
            nc.scalar.copy(e3s[:, HB:BL], e3ps[:, HB:BL])

            # ---- A^T pair-blocks. Pairs 0..NP-1-N_SBUF_PAIRS: TT straight
            # from PSUM (1x). Last N_SBUF_PAIRS pairs: ACT copies the PSUM
            # pair to SBUF bf16, TT then runs 2x. All E2 matmuls + TTs are
            # emitted before the C matmuls (PE FIFO is in-order; feeding the
            # DVE has priority). ----
            ats = []
            for p in range(NP):
                e2ps = psB.tile([128, 2, BL], F32, tag="e2")  # 2 PSUM banks
                for h in range(2):
                    q = 2 * p + h
                    nc.tensor.matmul(
                        e2ps[:, h, :], s2[:, 128 * q : 128 * (q + 1)], g2t,
                        start=True, stop=True,
                    )
                at = apool.tile([128, 2, BL], DBF, tag=f"at_{p}")
                e3b = e3s[:].unsqueeze(1).broadcast_to((128, 2, BL))
                if p >= NP - N_SBUF_PAIRS:
                    e2s = apool.tile([128, 2, BL], DBF, tag=f"e2s_{p}")
                    nc.scalar.copy(e2s[:], e2ps[:])
                    nc.vector.tensor_tensor(at[:], e2s[:], e3b, AluOpType.mult)
                else:
                    nc.vector.tensor_tensor(at[:], e2ps[:], e3b, AluOpType.mult)
                ats.append(at)

            # ---- g1 sigmoid (off the critical path; also frees gps so the
            # psA rotation can hand its bank to cps) ----
            nc.scalar.activation(g1t[:], gps[64:96, :], sig, scale=BETA)

            # ---- C accumulation over the 8 blocks ----
            cps = psA.tile([N, BL], F32, tag="ps")
            for q in range(NQ):
                p, h = q // 2, q % 2
                nc.tensor.matmul(
                    cps[:], vts[:, 32 * q : 32 * (q + 1)], ats[p][:, h, :],
                    start=(q == 0), stop=(q == NQ - 1),
                )

            # ---- out = ones.T @ (g1t .* C^T); final PSUM->SBUF copy split
            # ACT/DVE (both idle) to shorten the tail ----
            y = wpool.tile([N, BL], DBF)
            nc.vector.tensor_tensor(y[:], cps[:], g1t[:], AluOpType.mult)
            ops = psA.tile([1, BL], F32, tag="ps")
            nc.tensor.matmul(ops[:], ones, y[:], start=True, stop=True)
            outs = wpool.tile([1, BL], F32)
            nc.scalar.copy(outs[:, 0:HB], ops[:, 0:HB])
            nc.vector.tensor_copy(outs[:, HB:BL], ops[:, HB:BL])
            nc.sync.dma_start(out_d[:], outs[:])

    nc.finalize()
    return nc


def host_prep(x, W1, W2, W3, V):
    """Build per-core input maps (all numpy, fp32 in / packed layouts out)."""
    x = np.asarray(x, dtype=np.float32)
    W1 = np.asarray(W1, dtype=np.float32)
    W2 = np.asarray(W2, dtype=np.float32)
    W3 = np.asarray(W3, dtype=np.float32)
    V = np.asarray(V, dtype=np.float32)

    xT = np.ascontiguousarray(x.T)  # [128, 4096]

    # order: g2 rows first (E2-mm rhs at base partition 0), then g3 (base 32,
    # matching the S3 placement), then g1 (only needed at the very end)
    Wall = np.concatenate([W2, W3, W1], axis=0)  # [96, 128]
    cf = np.ascontiguousarray(Wall.T)  # [128, 96] fp32

    # V^T chunks: VTs[p, 32q + i] = V[0, i, j, k] with jk = 128q + p
    Vr = V.reshape(N, N * N)  # [i, jk]
    VT = np.ascontiguousarray(Vr.T)  # [jk, i]
    VTs = VT.reshape(NQ, 128, N).transpose(1, 0, 2).reshape(128, NQ * N)

    # E2 selection: S2[j', q*128 + p] = 1 iff j' == 4q + p//32
    S2 = np.zeros((N, NQ, 128), dtype=np.float32)
    for q in range(NQ):
        for p in range(128):
            S2[4 * q + p // 32, q, p] = 1.0
    S2pack = S2.reshape(N, NQ * 128)

    # E3 selection: S3[k', p] = 1 iff k' == p % 32
    S3 = np.zeros((N, 128), dtype=np.float32)
    for p in range(128):
        S3[p % 32, p] = 1.0

    cb = np.zeros((128, 384), dtype=BF16)
    cb[:, VT0:VT1] = VTs.astype(BF16)
    cb[32:64, S30:S31] = S3.astype(BF16)

    s2 = np.zeros((32, 1025), dtype=BF16)
    s2[:, S20:S21] = S2pack.astype(BF16)
    s2[:, ON0] = np.ones(N, dtype=BF16)

    wh = cf.astype(BF16)
    wl = (cf - wh.astype(np.float32)).astype(BF16)

    xa = np.zeros((128, XA1), dtype=BF16)
    xa[:, WH0:WH1] = wh
    xa[:, WL0:WL1] = wl

    in_maps = []
    for c in range(NCORES):
        m = xa.copy()
        xs = xT[:, c * BL : (c + 1) * BL]
        xhc = xs.astype(BF16)
        m[:, XH0:XH1] = xhc
        xlc = (xs - xhc.astype(np.float32)).astype(BF16)
        in_maps.append(
            {
                "xa0": np.ascontiguousarray(m[0:64]),
                "xa1": np.ascontiguousarray(m[64:128]),
                "xb0": np.ascontiguousarray(xlc[0:64]),
                "xb1": np.ascontiguousarray(xlc[64:128]),
                "cb": cb,
                "s2": s2,
            }
        )
    return in_maps


_CACHED_NC = None


def _ensure_ntff_hook():
    """The agent image's `antenv` package lacks `axon_hooks`; synthesize it
    and register the boot module's ctypes-based NTFF profile hook so
    run_bass_kernel_spmd(trace=True) can capture neuron-profile output."""
    import sys, types

    try:
        from antenv.axon_hooks import get_axon_ntff_profile_hook  # noqa: F401

        return
    except ImportError:
        pass
    import antenv
    from trn_agent_boot.trn_boot import _ntff_profile_via_ctypes

    mod = types.ModuleType("antenv.axon_hooks")
    mod._hook = _ntff_profile_via_ctypes("/opt/axon/libaxon_pjrt.so")
    mod.get_axon_ntff_profile_hook = lambda: mod._hook
    mod.set_axon_ntff_profile_hook = lambda h: setattr(mod, "_hook", h)
    sys.modules["antenv.axon_hooks"] = mod
    antenv.axon_hooks = mod


def run(inputs, trace=False, **trace_kwargs):
    """Run the kernel on 8 cores. Returns (out [4096] f32, BassKernelResults)."""
    global _CACHED_NC
    if trace:
        _ensure_ntff_hook()
    if _CACHED_NC is None:
        _CACHED_NC = build_nc()
    in_maps = host_prep(
        inputs["x"], inputs["W1"], inputs["W2"], inputs["W3"], inputs["V"]
    )
    res = run_bass_kernel_spmd(
        _CACHED_NC, in_maps, core_ids=list(range(NCORES)), trace=trace, **trace_kwargs
    )
    out = np.concatenate(
        [np.asarray(res.results[c]["out"]).reshape(BL) for c in range(NCORES)]
    ).astype(np.float32)
    return out, res


def kernel(**inputs):
    out, _ = run(inputs, trace=False)
    return out


# revision 5
# speedup vs baseline: 1.0047x; 1.0047x over previous
"""Trainium2 Bass kernel for nn_DLGN_VT (deep linearly-gated network w/ value tensor).

Math (per batch row b):
    g_i = sigmoid(30 * x @ W_i.T)            i = 1,2,3    [B, 32] each
    out[b] = sum_{ijk} g1[b,i] g2[b,j] g3[b,k] V[i,j,k]

Distribution: pure data-parallel over the batch axis, 8 NeuronCores,
512 rows per core. W_i and V are tiny and replicated.

Per-core v3 schedule (v1 26.0us, v2 25.4us):
  - Inputs split across the 3 DMA queues (SP + ACT HW-DGE, Pool SW-DGE),
    payload-balanced (~100GB/s each, ~0.8-1.4us ring latency), critical
    tensors first per queue:
      SP:  xa[0:64] (xh|wh)   xw[0:48] (xl|wl)   s2
      ACT: xa[64:128]         xw[48:96]
      PL:  xw[96:128]         cb (V^T | S3)
  - PE is kept gap-free from t~7.3us so the HAM clock-gate flips to 2.4GHz
    before the E2/C phase (v2 ran the whole middle at 1.2GHz): ~21 short
    N=128 warmup matmuls during the DMA wait, plus dummy matmuls into the
    first E2 PSUM pair during the xw and sigmoid waits.
  - Error-compensated bf16 gating (Wh.xh first, then Wl.xh + Wh.xl which
    need xw): Gps[96, 512] fp32-grade logits.
  - sigmoid -> g2t/g3t bf16; E3 = S3.T@g3t; e3s bf16 via a single DVE cast
    (an ACT-half copy kept getting scheduled behind g1's sigmoid).
  - A^T pair-blocks: E2 selection matmuls -> PSUM; pairs 0,1: DVE TT from
    PSUM (1x); pairs 2,3: ACT copies the pair to SBUF bf16 so the TT runs
    in 2x mode (~690 vs 1224ns).
  - C^T accumulates over 8 bf16 matmuls; out = ones.T @ (g1t .* C^T);
    final copy split ACT/DVE; single-packet output DMA.
"""

import numpy as np
import ml_dtypes

import concourse.bass as bass
import concourse.bacc as bacc
import concourse.mybir as mybir
import concourse.tile as tile
from concourse.alu_op_type import AluOpType
from concourse.bass_utils import run_bass_kernel_spmd

BF16 = ml_dtypes.bfloat16
NCORES = 8
B, D, N = 4096, 128, 32
BL = B // NCORES  # 512 batch rows per core
BETA = 30.0
NQ = 8   # 128-row blocks of the jk=1024 plane
NP = 4   # pairs of blocks

F32 = mybir.dt.float32
DBF = mybir.dt.bfloat16

# xa: bf16 [128, 608] = xh | wh   (bf16 hi of the xT shard / of Wall^T,
# order W2;W3;W1).  xw: bf16 [128, 608] = xl | wl (the lo halves).
XH0, XH1 = 0, BL
WH0, WH1 = XH1, XH1 + 96
XA1 = WH1  # 608
# cb: bf16 [128, 384]: V^T chunks [128, 256] | S3 at rows 32:64, cols 256:384
VT0, VT1 = 0, 256
S30, S31 = 256, 384
# s2: bf16 [32, 1025]: S2 selections [32, 1024] | ones [32, 1]
S20, S21 = 0, 1024
ON0 = 1024

N_WARMUP = 21    # short N=128 matmuls; ~107ns granularity during DMA wait
NW = 128         # warmup/dummy matmul free dim
N_DUMMY_XW = 2   # PE gap fillers while waiting for xw (gating passes 2-3)
N_DUMMY_SIG = 6  # PE gap fillers while waiting for the sigmoid
N_SBUF_PAIRS = 2  # pairs whose TT runs 2x from ACT-copied SBUF bf16


def build_nc():
    # Bacc (not raw Bass): its compile passes split multi-wait sync infos
    # (TRN2 allows at most one sync wait per compute instruction).
    nc = bacc.Bacc(None)
    xa0_d = nc.declare_dram_parameter("xa0", [64, XA1], DBF, isOutput=False)
    xa1_d = nc.declare_dram_parameter("xa1", [64, XA1], DBF, isOutput=False)
    xw0_d = nc.declare_dram_parameter("xw0", [48, XA1], DBF, isOutput=False)
    xw1_d = nc.declare_dram_parameter("xw1", [48, XA1], DBF, isOutput=False)
    xw2_d = nc.declare_dram_parameter("xw2", [32, XA1], DBF, isOutput=False)
    cb_d = nc.declare_dram_parameter("cb", [128, 384], DBF, isOutput=False)
    s2_d = nc.declare_dram_parameter("s2", [32, 1025], DBF, isOutput=False)
    out_d = nc.declare_dram_parameter("out", [1, BL], F32, isOutput=True)

    sig = mybir.ActivationFunctionType.Sigmoid

    with tile.TileContext(nc) as tc:
        with (
            tc.tile_pool(name="const", bufs=1) as cpool,
            tc.tile_pool(name="work", bufs=1) as wpool,
            tc.tile_pool(name="atp", bufs=1) as apool,
            tc.tile_pool(name="psA", bufs=2, space="PSUM") as psA,
            tc.tile_pool(name="psB", bufs=3, space="PSUM") as psB,
        ):
            xa = cpool.tile([128, XA1], DBF)
            xw = cpool.tile([128, XA1], DBF)
            cb = cpool.tile([128, 384], DBF)
            s2t = cpool.tile([32, 1025], DBF)

            # ---- input DMAs on three parallel queues, payload-balanced,
            # critical (gating) tensors first on each ----
            nc.sync.dma_start(xa[0:64, :], xa0_d[:])
            nc.scalar.dma_start(xa[64:128, :], xa1_d[:])
            nc.gpsimd.dma_start(xw[96:128, :], xw2_d[:])
            nc.sync.dma_start(xw[0:48, :], xw0_d[:])
            nc.scalar.dma_start(xw[48:96, :], xw1_d[:])
            nc.gpsimd.dma_start(cb[:], cb_d[:])
            nc.sync.dma_start(s2t[:], s2_d[:])

            xh = xa[:, XH0:XH1]
            wh = xa[:, WH0:WH1]
            xl = xw[:, XH0:XH1]
            wl = xw[:, WH0:WH1]
            vts = cb[:, VT0:VT1]             # [128, 8*32] V^T chunks (C lhsT)
            s3 = cb[32:64, S30:S31]          # [32, 128] E3 selection (base 32)
            s2 = s2t[:, S20:S21]             # [32, 8*128] E2 selection blocks
            ones = s2t[:, ON0 : ON0 + 1]     # [32, 1]

            # ---- PE warmup in the gating PSUM bank (overwritten later).
            # Short matmuls: fine-grained HAM busy-keeping, small overshoot
            # when the xa DMA lands. memset on DVE (idle until the cast). ----
            gps = psA.tile([96, BL], F32, tag="ps")
            wz = wpool.tile([128, NW], DBF)
            nc.vector.memset(wz[:], 0.0)
            for _ in range(N_WARMUP):
                nc.tensor.matmul(gps[:, 0:NW], wz[:, 0:96], wz[:],
                                 start=True, stop=True)

            # first E2 pair allocated early: its PSUM doubles as the dummy
            # gap-filler target (overwritten by the real E2 matmuls later)
            e2ps0 = psB.tile([128, 2, BL], F32, tag="e2")

            # ---- gating: error-compensated bf16 matmul; pass 1 needs only
            # xa, passes 2-3 wait for xw (dummies keep the PE busy) ----
            nc.tensor.matmul(gps[:], wh, xh, start=True, stop=False)
            for _ in range(N_DUMMY_XW):
                nc.tensor.matmul(e2ps0[:, 0, 0:NW], wz[:], wz[:],
                                 start=True, stop=True)
            nc.tensor.matmul(gps[:], wl, xh, start=False, stop=False)
            nc.tensor.matmul(gps[:], wh, xl, start=False, stop=True)

            g23 = wpool.tile([2 * N, BL], DBF)
            g1t = wpool.tile([N, BL], F32)
            nc.scalar.activation(g23[:], gps[0:64, :], sig, scale=BETA)
            g2t = g23[0:32, :]
            g3t = g23[32:64, :]  # base partition 32, matching s3

            # PE busy-keeping while the sigmoid runs
            for _ in range(N_DUMMY_SIG):
                nc.tensor.matmul(e2ps0[:, 0, 0:NW], wz[:], wz[:],
                                 start=True, stop=True)

            # ---- E3 = S3.T @ g3t -> e3s bf16 via one DVE cast ----
            e3ps = psA.tile([128, BL], F32, tag="ps")
            nc.tensor.matmul(e3ps[:], s3, g3t, start=True, stop=True)
            e3s = wpool.tile([128, BL], DBF)
            nc.vector.tensor_copy(e3s[:], e3ps[:])

            # ---- A^T pair-blocks. Pairs 0..NP-1-N_SBUF_PAIRS: TT straight
            # from PSUM (1x). Last N_SBUF_PAIRS pairs: ACT copies the PSUM
            # pair to SBUF bf16, TT then runs 2x. All E2 matmuls + TTs are
            # emitted before the C matmuls (PE FIFO is in-order; feeding the
            # DVE has priority). ----
            ats = []
            for p in range(NP):
                e2ps = e2ps0 if p == 0 else psB.tile([128, 2, BL], F32, tag="e2")
                for h in range(2):
                    q = 2 * p + h
                    nc.tensor.matmul(
                        e2ps[:, h, :], s2[:, 128 * q : 128 * (q + 1)], g2t,
                        start=True, stop=True,
                    )
                at = apool.tile([128, 2, BL], DBF, tag=f"at_{p}")
                e3b = e3s[:].unsqueeze(1).broadcast_to((128, 2, BL))
                if p >= NP - N_SBUF_PAIRS:
                    e2s = apool.tile([128, 2, BL], DBF, tag=f"e2s_{p}")
                    nc.scalar.copy(e2s[:], e2ps[:])
                    nc.vector.tensor_tensor(at[:], e2s[:], e3b, AluOpType.mult)
                else:
                    nc.vector.tensor_tensor(at[:], e2ps[:], e3b, AluOpType.mult)
                ats.append(at)

            # ---- g1 sigmoid (off the critical path; also frees gps so the
            # psA rotation can hand its bank to cps) ----
            nc.scalar.activation(g1t[:], gps[64:96, :], sig, scale=BETA)

            # ---- C accumulation over the 8 blocks ----
            cps = psA.tile([N, BL], F32, tag="ps")
            for q in range(NQ):
                p, h = q // 2, q % 2
                nc.tensor.matmul(
                    cps[:], vts[:, 32 * q : 32 * (q + 1)], ats[p][:, h, :],
                    start=(q == 0), stop=(q == NQ - 1),
                )

            # ---- out = ones.T @ (g1t .* C^T); final PSUM->SBUF copy split
            # ACT/DVE (both idle) to shorten the tail ----
            y = wpool.tile([N, BL], DBF)
            nc.vector.tensor_tensor(y[:], cps[:], g1t[:], AluOpType.mult)
            ops = psA.tile([1, BL], F32, tag="ps")
            nc.tensor.matmul(ops[:], ones, y[:], start=True, stop=True)
            outs = wpool.tile([1, BL], F32)
            HB = BL // 2
            nc.scalar.copy(outs[:, 0:HB], ops[:, 0:HB])
            nc.vector.tensor_copy(outs[:, HB:BL], ops[:, HB:BL])
            nc.sync.dma_start(out_d[:], outs[:], single_packet=True)

    nc.finalize()
    return nc


def host_prep(x, W1, W2, W3, V):
    """Build per-core input maps (all numpy, fp32 in / packed layouts out)."""
    x = np.asarray(x, dtype=np.float32)
    W1 = np.asarray(W1, dtype=np.float32)
    W2 = np.asarray(W2, dtype=np.float32)
    W3 = np.asarray(W3, dtype=np.float32)
    V = np.asarray(V, dtype=np.float32)

    xT = np.ascontiguousarray(x.T)  # [128, 4096]

    # order: g2 rows first (E2-mm rhs at base partition 0), then g3 (base 32,
    # matching the S3 placement), then g1 (only needed at the very end)
    Wall = np.concatenate([W2, W3, W1], axis=0)  # [96, 128]
    cf = np.ascontiguousarray(Wall.T)  # [128, 96] fp32

    # V^T chunks: VTs[p, 32q + i] = V[0, i, j, k] with jk = 128q + p
    Vr = V.reshape(N, N * N)  # [i, jk]
    VT = np.ascontiguousarray(Vr.T)  # [jk, i]
    VTs = VT.reshape(NQ, 128, N).transpose(1, 0, 2).reshape(128, NQ * N)

    # E2 selection: S2[j', q*128 + p] = 1 iff j' == 4q + p//32
    S2 = np.zeros((N, NQ, 128), dtype=np.float32)
    for q in range(NQ):
        for p in range(128):
            S2[4 * q + p // 32, q, p] = 1.0
    S2pack = S2.reshape(N, NQ * 128)

    # E3 selection: S3[k', p] = 1 iff k' == p % 32
    S3 = np.zeros((N, 128), dtype=np.float32)
    for p in range(128):
        S3[p % 32, p] = 1.0

    cb = np.zeros((128, 384), dtype=BF16)
    cb[:, VT0:VT1] = VTs.astype(BF16)
    cb[32:64, S30:S31] = S3.astype(BF16)

    s2 = np.zeros((32, 1025), dtype=BF16)
    s2[:, S20:S21] = S2pack.astype(BF16)
    s2[:, ON0] = np.ones(N, dtype=BF16)

    wh = cf.astype(BF16)
    wl = (cf - wh.astype(np.float32)).astype(BF16)

    in_maps = []
    for c in range(NCORES):
        xs = xT[:, c * BL : (c + 1) * BL]
        xhc = xs.astype(BF16)
        xlc = (xs - xhc.astype(np.float32)).astype(BF16)
        xa = np.zeros((128, XA1), dtype=BF16)
        xa[:, XH0:XH1] = xhc
        xa[:, WH0:WH1] = wh
        xwm = np.zeros((128, XA1), dtype=BF16)
        xwm[:, XH0:XH1] = xlc
        xwm[:, WH0:WH1] = wl
        in_maps.append(
            {
                "xa0": np.ascontiguousarray(xa[0:64]),
                "xa1": np.ascontiguousarray(xa[64:128]),
                "xw0": np.ascontiguousarray(xwm[0:48]),
                "xw1": np.ascontiguousarray(xwm[48:96]),
                "xw2": np.ascontiguousarray(xwm[96:128]),
                "cb": cb,
                "s2": s2,
            }
        )
    return in_maps


_CACHED_NC = None


def _ensure_ntff_hook():
    """The agent image's `antenv` package lacks `axon_hooks`; synthesize it
    and register the boot module's ctypes-based NTFF profile hook so
    run_bass_kernel_spmd(trace=True) can capture neuron-profile output."""
    import sys, types

    try:
        from antenv.axon_hooks import get_axon_ntff_profile_hook  # noqa: F401

        return
    except ImportError:
        pass
    import antenv
    from trn_agent_boot.trn_boot import _ntff_profile_via_ctypes

    mod = types.ModuleType("antenv.axon_hooks")
    mod._hook = _ntff_profile_via_ctypes("/opt/axon/libaxon_pjrt.so")
    mod.get_axon_ntff_profile_hook = lambda: mod._hook
    mod.set_axon_ntff_profile_hook = lambda h: setattr(mod, "_hook", h)
    sys.modules["antenv.axon_hooks"] = mod
    antenv.axon_hooks = mod


def run(inputs, trace=False, **trace_kwargs):
    """Run the kernel on 8 cores. Returns (out [4096] f32, BassKernelResults)."""
    global _CACHED_NC
    if trace:
        _ensure_ntff_hook()
    if _CACHED_NC is None:
        _CACHED_NC = build_nc()
    in_maps = host_prep(
        inputs["x"], inputs["W1"], inputs["W2"], inputs["W3"], inputs["V"]
    )
    res = run_bass_kernel_spmd(
        _CACHED_NC, in_maps, core_ids=list(range(NCORES)), trace=trace, **trace_kwargs
    )
    out = np.concatenate(
        [np.asarray(res.results[c]["out"]).reshape(BL) for c in range(NCORES)]
    ).astype(np.float32)
    return out, res


def kernel(**inputs):
    out, _ = run(inputs, trace=False)
    return out
